# revision 1
# baseline (speedup 1.0000x reference)
import sys

if "/opt/trn_rl_repo" not in sys.path:
    sys.path.insert(0, "/opt/trn_rl_repo")

import numpy as np

import concourse.bacc as bacc
import concourse.mybir as mybir
import concourse.tile as tile
from concourse.bass_utils import run_bass_kernel_spmd

dt = mybir.dt
Alu = mybir.AluOpType

TOKENS = 8192
IN_FEATURES = 4096
OUT_FEATURES = 4096

N_CORES = 8
T_SHARD = 2
F_SHARD = 4

TOK = TOKENS // T_SHARD
K = IN_FEATURES
FPC = OUT_FEATURES // F_SHARD
KB = K // 2
KC = K // 128
FT = FPC // 128
MT = TOK // 128
NG = FPC // 512
DQ_CH = 2
KC2 = 16
KC1 = KC - KC2


def _emit_dequant(nc, b, pp, sh, c, wout, c16, cones):
    v = nc.vector
    ih = pp
    v.tensor_scalar(ih, b, 0x70, 2, Alu.bitwise_and, Alu.logical_shift_left)
    il = c.bitcast(dt.int32)[:, : b.shape[1]]
    v.tensor_scalar(il, b, 0x07, 6, Alu.bitwise_and, Alu.logical_shift_left)
    v.tensor_scalar(ih, ih, 0x3F00, None, Alu.add)
    v.tensor_scalar(il, il, 0x3F00, None, Alu.add)
    v.scalar_tensor_tensor(pp, il, c16[:], ih, Alu.logical_shift_left, Alu.bitwise_or)
    v.tensor_scalar(sh, b, 128, 8, Alu.bitwise_and, Alu.logical_shift_left)
    v.tensor_scalar(b, b, 8, 28, Alu.bitwise_and, Alu.logical_shift_left)
    v.scalar_tensor_tensor(sh, sh, cones[:], b, Alu.bitwise_or, Alu.bitwise_or)
    fp = pp.bitcast(dt.bfloat16)
    fs = sh.bitcast(dt.bfloat16)
    v.tensor_scalar(c, fp, 1.0, 0.0, Alu.subtract, Alu.min)
    v.tensor_tensor(fp, fp, c, Alu.add)
    v.tensor_tensor(wout, fp, fs, Alu.mult)


def build(reps=1):
    nc = bacc.Bacc()
    x_d = nc.dram_tensor("x", [TOK, K], dt.float32, kind="ExternalInput")
    wp_d = nc.dram_tensor("wp", [FPC, KB], dt.int32, kind="ExternalInput")
    ws_d = nc.dram_tensor("ws", [1], dt.float32, kind="ExternalInput")
    bias_d = nc.dram_tensor("bias", [FPC], dt.float32, kind="ExternalInput")
    out_d = nc.dram_tensor("out", [TOK, FPC], dt.float32, kind="ExternalOutput")

    CH = KB // DQ_CH

    with tile.TileContext(nc) as tc:
        with (
            tc.tile_pool(name="const", bufs=1) as const,
            tc.tile_pool(name="wdq", bufs=1) as wdq_pool,
            tc.tile_pool(name="xpool", bufs=2) as xpool,
            tc.tile_pool(name="opool", bufs=4) as opool,
            tc.tile_pool(name="psum", bufs=4, space="PSUM") as psum_pool,
        ):
            c16 = const.tile([128, 1], dt.int32)
            nc.vector.memset(c16[:], 16)
            cones = const.tile([128, 1], dt.int32)
            nc.vector.memset(cones[:], 0x3F803F80)

            scol = const.tile([128, 1], dt.float32)
            nc.sync.dma_start(
                scol[:], ws_d[:].rearrange("(a s) -> a s", a=1).to_broadcast([128, 1])
            )
            bt = const.tile([128, FPC], dt.float32)
            nc.sync.dma_start(
                bt[:],
                bias_d[:].rearrange("(a f) -> a f", a=1).to_broadcast([128, FPC]),
            )

            for _rep in range(reps):
                wt = const.tile([128, FT, KC, 128], dt.bfloat16)
                for ft in range(FT):
                    wbf = wdq_pool.tile([128, K], dt.bfloat16, name="wbf", bufs=4)
                    for ch in range(DQ_CH):
                        b = wdq_pool.tile([128, CH], dt.int32, name="b", bufs=2)
                        nc.sync.dma_start(
                            b[:],
                            wp_d[ft * 128 : (ft + 1) * 128, ch * CH : (ch + 1) * CH],
                        )
                        pp = wdq_pool.tile([128, CH], dt.int32, name="pp")
                        sh = wdq_pool.tile([128, CH], dt.int32, name="sh")
                        c = wdq_pool.tile([128, CH * 2], dt.bfloat16, name="c")
                        _emit_dequant(
                            nc,
                            b[:],
                            pp[:],
                            sh[:],
                            c[:],
                            wbf[:, ch * 2 * CH : (ch + 1) * 2 * CH],
                            c16,
                            cones,
                        )
                    nc.scalar.dma_start_transpose(
                        wt[:, ft, :KC1, :], wbf[:, : KC1 * 128]
                    )
                    nc.scalar.dma_start_transpose(
                        wt[:, ft, KC1:, :], wbf[:, KC1 * 128 :]
                    )

                wt8 = const.tile([128, KC2, FPC], dt.float8e4)
                for ft in range(FT):
                    nc.scalar.copy(
                        wt8[:, :, ft * 128 : (ft + 1) * 128],
                        wt[:, ft, KC1:, :],
                    )

                for m in range(MT):
                    xb = xpool.tile([128, K], dt.bfloat16, name="xb", bufs=3)
                    nc.gpsimd.dma_start(xb[:], x_d[m * 128 : (m + 1) * 128, :])
                    xt = xpool.tile([128, KC, 128], dt.bfloat16, name="xt", bufs=4)
                    nc.scalar.copy(xt[0:1, 0:1, 0:1], xb[0:1, 0:1])
                    nc.scalar.dma_start_transpose(xt[:], xb[:])
                    xt8 = xpool.tile([128, KC2, 128], dt.float8e4, name="xt8", bufs=3)
                    nc.vector.tensor_copy(xt8[:], xt[:, KC1:, :])

                    for g in range(NG):
                        ps = psum_pool.tile([128, 512], dt.float32)
                        for kc in range(KC1):
                            nc.tensor.matmul(
                                ps[:],
                                xt[:, kc, :],
                                wt[:, 4 * g : 4 * (g + 1), kc, :],
                                start=(kc == 0),
                                stop=False,
                            )
                        for c in range(KC2 // 2):
                            nc.tensor.matmul(
                                ps[:],
                                xt8[:, 2 * c : 2 * c + 2, :],
                                wt8[:, 2 * c : 2 * c + 2, g * 512 : (g + 1) * 512],
                                start=False,
                                stop=(c == KC2 // 2 - 1),
                                perf_mode=mybir.MatmulPerfMode.DoubleRow,
                            )
                        osb = opool.tile([128, 512], dt.float32, name="osb")
                        nc.vector.scalar_tensor_tensor(
                            osb[:],
                            ps[:],
                            scol[:],
                            bt[:, g * 512 : (g + 1) * 512],
                            Alu.mult,
                            Alu.add,
                        )
                        (nc.sync if (m >= MT - 2 or m < 2) else nc.scalar).dma_start(
                            out_d[m * 128 : (m + 1) * 128, g * 512 : (g + 1) * 512],
                            osb[:],
                        )
    nc.finalize()
    return nc


_NC = None


def _get_nc():
    global _NC
    if _NC is None:
        _NC = build()
    return _NC


def make_in_maps(x, weight_packed, weight_scale, bias):
    x = np.ascontiguousarray(np.asarray(x, dtype=np.float32))
    wp = np.asarray(weight_packed, dtype=np.int32).reshape(OUT_FEATURES, KB)
    ws = np.ascontiguousarray(np.asarray(weight_scale, dtype=np.float32))
    bias = np.asarray(bias, dtype=np.float32)
    in_maps = []
    for core in range(N_CORES):
        th, q = divmod(core, F_SHARD)
        in_maps.append(
            {
                "x": x[th * TOK : (th + 1) * TOK],
                "wp": np.ascontiguousarray(wp[q * FPC : (q + 1) * FPC]),
                "ws": ws,
                "bias": np.ascontiguousarray(bias[q * FPC : (q + 1) * FPC]),
            }
        )
    return in_maps


def unshard(results):
    out = np.empty((TOKENS, OUT_FEATURES), dtype=np.float32)
    for core in range(N_CORES):
        th, q = divmod(core, F_SHARD)
        out[th * TOK : (th + 1) * TOK, q * FPC : (q + 1) * FPC] = results[core]["out"]
    return out


def run(inputs, **kwargs):
    nc = _get_nc()
    res = run_bass_kernel_spmd(
        nc, make_in_maps(**inputs), core_ids=list(range(N_CORES)), **kwargs
    )
    return unshard(res.results), res


def kernel(x, weight_packed, weight_scale, bias):
    out, _ = run(
        {
            "x": x,
            "weight_packed": weight_packed,
            "weight_scale": weight_scale,
            "bias": bias,
        }
    )
    return out


if __name__ == "__main__":
    rng = np.random.default_rng(0)
    inputs = {
        "x": rng.standard_normal((TOKENS, IN_FEATURES), dtype=np.float32),
        "weight_packed": rng.integers(
            0, 256, size=OUT_FEATURES * IN_FEATURES // 2
        ).astype(np.int32),
        "weight_scale": rng.random(1, dtype=np.float32),
        "bias": rng.standard_normal(OUT_FEATURES).astype(np.float32),
    }
    out = kernel(**inputs)
    print("out", out.shape, out.dtype, out[0, :4])

